# revision 60
# baseline (speedup 1.0000x reference)
"""GCN layer (GCNConv + ReLU) Bass kernel for 8 Trainium2 NeuronCores. v2

Reference computation (PyG GCNConv with self-loops, eval mode):
    deg  = in-degree(dst) + 1                       (self loops included)
    norm_e = deg^-1/2[src_e] * deg^-1/2[dst_e]
    out  = relu( segment_sum_dst( x[src] * norm ) @ W + b )   (W applied last)

Device strategy (per core, SPMD over 8 cores):
  - dst nodes are greedily packed into chunks of <=125 nodes (128 slots),
    balancing BOTH the lo-src (src<32768) and hi-src in-edge counts so every
    chunk fits B_LO lo blocks + B_HI hi blocks of 128 edges.
  - x is stored in DRAM in bf16 (lo/hi split for the int16 gather index
    limit). Per 1024 edges one SWDGE dma_gather pulls rows into SBUF;
    gathers round-robin over 4 SWDGE queues (num_swdge_queues=4 plus a 64KB
    descriptor-ring carveout) so descriptor generation runs on all 4 Q7
    core pairs concurrently: ~2.2ns/idx vs ~8.5ns/idx on queue 0 alone.
  - The select matrices S[e, slot] = onehot(dst_e)*norm_e are built on the
    HOST (bf16, one [128, B*128] supertile per chunk) and streamed in by
    HWDGE DMA; keeping the DVE idle matters because DVE activity contends
    with Q7 descriptor generation for the shared SBUF port.  TensorE
    accumulates agg[cin, slot] += sum_e G[e, cin] * S[e, slot] in PSUM.
  - Per chunk: aggT @ W (bf16), bias+ReLU on ScalarE, store bf16
    [cout, slot]; host unpermutes/transposes to the full [N, C] f32 output.
"""

import heapq
import os

import ml_dtypes
import numpy as np

import concourse.bacc as bacc
import concourse.bass as bass
import concourse.mybir as mybir
import concourse.tile as tile
from concourse.bass_utils import run_bass_kernel_spmd

N_CORES = 8
CHUNK_W = 128  # dst slots per chunk
SPLIT = 32768  # gather table split point (int16 index limit)
NODES_PER_CHUNK = 125
NQ = 4  # SWDGE queues
# how many select blocks per chunk the DVE builds on-chip (is_equal*norm);
# the rest stream from host-built supertiles via DMA
L_DVE = int(os.environ.get("GCN_LDVE", "0"))

LAST_RUN_INFO = {}
BF16 = ml_dtypes.bfloat16


def _host_prep(x, edge_index):
    """Host-side sharding: chunk assignment, edge bucketing, index layout."""
    N, C = x.shape
    src = np.asarray(edge_index[0], dtype=np.int64)
    dst = np.asarray(edge_index[1], dtype=np.int64)
    loops = np.arange(N, dtype=np.int64)
    src = np.concatenate([src, loops])
    dst = np.concatenate([dst, loops])

    deg = np.bincount(dst, minlength=N)
    dinv = (1.0 / np.sqrt(deg.astype(np.float64))).astype(np.float32)
    norm = (dinv[src] * dinv[dst]).astype(np.float32)

    lo_in = np.bincount(dst[src < SPLIT], minlength=N)
    hi_in = deg - lo_in

    nchunks = N_CORES * int(np.ceil(N / (N_CORES * NODES_PER_CHUNK)))
    cpc = nchunks // N_CORES

    # Greedy bi-criteria packing: balance lo and hi edge counts per chunk.
    chunk_of, load_lo, load_hi = _greedy_chunks(lo_in, hi_in, nchunks)
    # swap-repair towards the block-quantized ideal (saves a whole gather
    # block per chunk when the greedy max is just above a 128 boundary)
    t_lo = max(int(np.ceil(load_lo.sum() / nchunks / 128)) * 128, 128)
    t_hi = max(int(np.ceil(load_hi.sum() / nchunks / 128)) * 128, 128)
    if load_lo.max() > t_lo or load_hi.max() > t_hi:
        rep = _repair_chunks(chunk_of, load_lo, load_hi, lo_in, hi_in, t_lo, t_hi)
        if rep is not None:
            chunk_of, load_lo, load_hi = rep
    B_lo = int(np.ceil(load_lo.max() / 128))
    B_hi = int(np.ceil(load_hi.max() / 128))
    B = B_lo + B_hi

    # slot within chunk
    order = np.argsort(chunk_of, kind="stable")
    slot_of = np.empty(N, np.int64)
    slot_of[order] = np.arange(N) - np.searchsorted(
        chunk_of[order], chunk_of[order], side="left"
    )
    assert slot_of.max() < CHUNK_W

    # bucket edges by (chunk, hi) and rank within bucket
    e_chunk = chunk_of[dst]
    islo = src < SPLIT
    key = e_chunk * 2 + (~islo).astype(np.int64)
    perm = np.argsort(key, kind="stable")
    ks = key[perm]
    gsz = np.bincount(key, minlength=2 * nchunks)
    gstart = np.zeros(2 * nchunks, np.int64)
    gstart[1:] = np.cumsum(gsz)[:-1]
    rank = np.arange(len(ks)) - gstart[ks]

    S_lo, S_hi = B_lo * 128, B_hi * 128
    # per-chunk flat edge slots: lo region then hi region
    flat_idx = np.zeros(nchunks * (S_lo + S_hi), np.int64)  # pad idx -> 0
    flat_dst = np.full(nchunks * (S_lo + S_hi), -1.0, np.float32)
    flat_nrm = np.zeros(nchunks * (S_lo + S_hi), np.float32)
    cc = ks // 2
    s = ks % 2
    col = cc * (S_lo + S_hi) + np.where(s == 0, rank, S_lo + rank)
    ss = src[perm]
    flat_idx[col] = np.where(s == 0, ss, ss - SPLIT)
    flat_dst[col] = slot_of[dst[perm]].astype(np.float32)
    flat_nrm[col] = norm[perm]
    assert flat_idx.max() < SPLIT and flat_idx.min() >= 0
    flat_idx = flat_idx.astype(np.int16)

    A_idx = flat_idx.reshape(nchunks, S_lo + S_hi)
    A_dst = flat_dst.reshape(nchunks, (S_lo + S_hi) // 128, 128)
    A_nrm = flat_nrm.reshape(nchunks, (S_lo + S_hi) // 128, 128)

    # Select matrices: blocks [0, L) of each chunk are built on-chip by the
    # DVE from (dstslot, norm) scalars; blocks [L, B) are host-built bf16
    # supertiles S[p, (chunk*(B-L) + block-L)*128 + dstslot] = norm.
    L = min(L_DVE, B)
    ch_i, blk_i, p_i = np.nonzero(A_dst >= 0)
    dma_m = blk_i >= L
    slot_i = A_dst[ch_i, blk_i, p_i].astype(np.int64)
    nrm_i = A_nrm[ch_i, blk_i, p_i].astype(BF16)
    col_i = (ch_i % cpc) * ((B - L) * 128) + (blk_i - L) * 128 + slot_i

    per_core = []
    for k in range(N_CORES):
        sl = slice(k * cpc, (k + 1) * cpc)
        ilo = A_idx[sl, :S_lo].reshape(-1)  # cpc*B_lo*128, stream order
        ihi = A_idx[sl, S_lo:].reshape(-1)
        m = (ch_i >= k * cpc) & (ch_i < (k + 1) * cpc) & dma_m
        seldata = np.zeros((128, cpc * (B - L) * 128), BF16)
        seldata[p_i[m], col_i[m]] = nrm_i[m]
        per_core.append(
            dict(
                idx_lo=_wrap_idx(ilo),
                idx_hi=_wrap_idx(ihi),
                seldata=seldata,
                dstslot=np.ascontiguousarray(
                    A_dst[sl, :L, :].reshape(cpc * L, 128).T
                ),
                normv=np.ascontiguousarray(
                    A_nrm[sl, :L, :].reshape(cpc * L, 128).T
                ),
            )
        )

    meta = dict(
        N=N,
        C=C,
        B_lo=B_lo,
        B_hi=B_hi,
        cpc=cpc,
        nchunks=nchunks,
        chunk_of=chunk_of,
        slot_of=slot_of,
    )
    return per_core, meta


def _greedy_chunks(lo_in, hi_in, nchunks):
    """Pack nodes into chunks (<=NODES_PER_CHUNK each), balancing lo and hi
    in-edge loads.  Returns chunk assignment and per-chunk loads."""
    mean_lo = lo_in.sum() / nchunks
    mean_hi = hi_in.sum() / nchunks
    for slack_lo, slack_hi in [
        (1.058, 1.046),
        (1.058, 1.14),
        (1.10, 1.22),
        (1.25, 1.40),
        (9.9, 9.9),
    ]:
        cap_lo = int(mean_lo * slack_lo)
        cap_hi = int(mean_hi * slack_hi)
        res = _greedy_try(lo_in, hi_in, nchunks, cap_lo, cap_hi)
        if res is not None:
            return res
    raise RuntimeError("chunk packing failed")


def _repair_chunks(chunk_of, load_lo, load_hi, lo_in, hi_in, t_lo, t_hi):
    """Local-search repair: swap nodes between chunks until every chunk's
    lo load <= t_lo and hi load <= t_hi.  Returns None if stuck (caller
    keeps the unrepaired assignment)."""
    if load_lo.sum() > t_lo * len(load_lo) or load_hi.sum() > t_hi * len(load_hi):
        return None  # targets infeasible
    chunk_of = chunk_of.copy()
    load_lo = load_lo.copy()
    load_hi = load_hi.copy()
    nch = len(load_lo)
    members = [[] for _ in range(nch)]
    for n in range(len(chunk_of)):
        members[chunk_of[n]].append(n)

    def fix(dim_in, other_in, load_d, load_o, t_d, t_o):
        for A in np.argsort(-load_d):
            A = int(A)
            guard = 0
            while load_d[A] > t_d and guard < 400:
                guard += 1
                a = max(members[A], key=lambda n: dim_in[n])
                done = False
                for B in np.argsort(load_d):
                    B = int(B)
                    if B == A:
                        continue
                    for b in sorted(members[B], key=lambda n: dim_in[n])[:24]:
                        ndA = load_d[A] - dim_in[a] + dim_in[b]
                        noA = load_o[A] - other_in[a] + other_in[b]
                        ndB = load_d[B] - dim_in[b] + dim_in[a]
                        noB = load_o[B] - other_in[b] + other_in[a]
                        if (
                            ndA < load_d[A]
                            and noA <= t_o
                            and ndB <= t_d
                            and noB <= t_o
                        ):
                            members[A].remove(a)
                            members[B].remove(b)
                            members[A].append(b)
                            members[B].append(a)
                            chunk_of[a], chunk_of[b] = B, A
                            load_d[A], load_o[A] = ndA, noA
                            load_d[B], load_o[B] = ndB, noB
                            done = True
                            break
                    if done:
                        break
                if not done:
                    return False
        return True

    if not fix(lo_in, hi_in, load_lo, load_hi, t_lo, t_hi):
        return None
    if not fix(hi_in, lo_in, load_hi, load_lo, t_hi, t_lo):
        return None
    if load_lo.max() > t_lo or load_hi.max() > t_hi:
        return None
    return chunk_of, load_lo, load_hi


def _greedy_try(lo_in, hi_in, nchunks, cap_lo, cap_hi):
    N = len(lo_in)
    order = np.argsort(-(lo_in / cap_lo + hi_in / cap_hi), kind="stable")
    load_lo = np.zeros(nchunks)
    load_hi = np.zeros(nchunks)
    cnt = np.zeros(nchunks, np.int64)
    chunk_of = np.empty(N, np.int64)
    h = [(0.0, c) for c in range(nchunks)]
    heapq.heapify(h)
    for n in order:
        popped = []
        while True:
            if not h:
                return None
            l, c = heapq.heappop(h)
            if (
                cnt[c] < NODES_PER_CHUNK
                and load_lo[c] + lo_in[n] <= cap_lo
                and load_hi[c] + hi_in[n] <= cap_hi
            ):
                break
            popped.append((l, c))
        for p in popped:
            heapq.heappush(h, p)
        chunk_of[n] = c
        cnt[c] += 1
        load_lo[c] += lo_in[n]
        load_hi[c] += hi_in[n]
        heapq.heappush(h, (max(load_lo[c] / cap_lo, load_hi[c] / cap_hi), c))
    return chunk_of, load_lo, load_hi


def _wrap_idx(flat):
    """Gather idx layout: per 1024-idx instruction, [i%16 (x8 replicated), i//16].
    flat is the concatenated block stream; padded to a multiple of 1024 with
    -1: the gather ucode trims trailing negative indices before descriptor
    generation, and these slots belong to no chunk so they are never read."""
    n = len(flat)
    ninst = int(np.ceil(n / 1024))
    pad = np.full(ninst * 1024, -1, flat.dtype)
    pad[:n] = flat
    cols = []
    for i in range(ninst):
        v = pad[i * 1024:(i + 1) * 1024].reshape(64, 16).T  # [16, 64]
        cols.append(np.tile(v, (8, 1)))
    return np.ascontiguousarray(np.concatenate(cols, axis=1))


def _build_program(N, C, B_lo, B_hi, cpc):
    f32 = mybir.dt.float32
    bf16 = mybir.dt.bfloat16
    i16 = mybir.dt.int16
    B = B_lo + B_hi
    L = min(L_DVE, B)
    n_lo_inst = (cpc * B_lo * 128 + 1023) // 1024
    n_hi_inst = (cpc * B_hi * 128 + 1023) // 1024

    nc = bacc.Bacc(None, target_bir_lowering=False, debug=False,
                   num_swdge_queues=NQ, dynamic_dma_scratch_size=65536)

    xlo_d = nc.dram_tensor("x_lo", [SPLIT, C], bf16, kind="ExternalInput")
    xhi_d = nc.dram_tensor("x_hi", [N - SPLIT, C], bf16, kind="ExternalInput")
    ilo_d = nc.dram_tensor("idx_lo", [128, n_lo_inst * 64], i16, kind="ExternalInput")
    ihi_d = nc.dram_tensor("idx_hi", [128, n_hi_inst * 64], i16, kind="ExternalInput")
    sel_d = nc.dram_tensor("seldata", [128, cpc * (B - L) * 128], bf16, kind="ExternalInput")
    if L:
        dst_d = nc.dram_tensor("dstslot", [128, cpc * L], f32, kind="ExternalInput")
        nrm_d = nc.dram_tensor("normv", [128, cpc * L], f32, kind="ExternalInput")
        iota_d = nc.dram_tensor("iota", [128, CHUNK_W], bf16, kind="ExternalInput")
    w_d = nc.dram_tensor("weight", [C, C], bf16, kind="ExternalInput")
    b_d = nc.dram_tensor("bias", [128, 1], f32, kind="ExternalInput")
    out_d = nc.dram_tensor("out", [128, cpc * CHUNK_W], bf16, kind="ExternalOutput")

    with tile.TileContext(nc) as tc:
        with (
            tc.tile_pool(name="const", bufs=1) as constp,
            tc.tile_pool(name="gat_lo", bufs=12) as glop,
            tc.tile_pool(name="gat_hi", bufs=8) as ghip,
            tc.tile_pool(name="sel", bufs=6) as selp,
            tc.tile_pool(name="dvesel", bufs=8) as dvesp,
            tc.tile_pool(name="aggs", bufs=4) as aggsp,
            tc.tile_pool(name="outs", bufs=4) as outsp,
            tc.tile_pool(name="pagg", bufs=4, space="PSUM") as pagg,
            tc.tile_pool(name="pout", bufs=2, space="PSUM") as pout,
        ):
            w_t = constp.tile([C, C], bf16, tag="w")
            nc.sync.dma_start(w_t[:], w_d[:])
            bias_t = constp.tile([128, 1], f32, tag="bias")
            nc.sync.dma_start(bias_t[:], b_d[:])
            if L:
                iota_t = constp.tile([128, CHUNK_W], bf16, tag="iota")
                nc.sync.dma_start(iota_t[:], iota_d[:])
                dst_t = constp.tile([128, cpc * L], f32, tag="dst")
                nc.sync.dma_start(dst_t[:], dst_d[:])
                nrm_t = constp.tile([128, cpc * L], f32, tag="nrm")
                nc.sync.dma_start(nrm_t[:], nrm_d[:])
            # tiny warmup gather: loads the Q7 dma_gather ucode (~6us IRAM
            # load) concurrently with the idx table DMAs
            warm_t = constp.tile([128, 1, C], bf16, tag="warm")
            warmidx_t = constp.tile([128, 8], i16, tag="warmidx")
            nc.gpsimd.memset(warmidx_t[:], 0)
            nc.gpsimd.dma_gather(warm_t[:], xlo_d[:], warmidx_t[:], 128, 128, C)
            ilo_t = constp.tile([128, n_lo_inst * 64], i16, tag="ilo")
            nc.sync.dma_start(ilo_t[:], ilo_d[:])
            ihi_t = constp.tile([128, n_hi_inst * 64], i16, tag="ihi")
            nc.sync.dma_start(ihi_t[:], ihi_d[:])

            qctr = [0]
            lo_tiles, hi_tiles = [], []

            def issue_lo():
                i = len(lo_tiles)
                gt = glop.tile([128, 8, C], bf16, tag="glo")
                nc.gpsimd.dma_gather(
                    gt[:], xlo_d[:], ilo_t[:, i * 64:(i + 1) * 64],
                    1024, 1024, C, queue_num=qctr[0] % NQ,
                )
                qctr[0] += 1
                lo_tiles.append(gt)

            def issue_hi():
                i = len(hi_tiles)
                gt = ghip.tile([128, 8, C], bf16, tag="ghi")
                nc.gpsimd.dma_gather(
                    gt[:], xhi_d[:], ihi_t[:, i * 64:(i + 1) * 64],
                    1024, 1024, C, queue_num=qctr[0] % NQ,
                )
                qctr[0] += 1
                hi_tiles.append(gt)

            def lo_block(g):
                while len(lo_tiles) * 8 <= g:
                    issue_lo()
                return lo_tiles[g // 8][:, g % 8, :]

            def hi_block(g):
                while len(hi_tiles) * 8 <= g:
                    issue_hi()
                return hi_tiles[g // 8][:, g % 8, :]

            for c in range(cpc):
                agg_t = pagg.tile([128, CHUNK_W], mybir.dt.float32, tag="agg")
                sel_t = selp.tile([128, (B - L) * 128], bf16, tag="sel")
                sel_eng = nc.sync if c % 2 == 0 else nc.scalar
                sel_eng.dma_start(
                    sel_t[:], sel_d[:, c * (B - L) * 128:(c + 1) * (B - L) * 128]
                )
                for b in range(B):
                    if b < L:
                        gb = c * L + b
                        dve_t = dvesp.tile([128, CHUNK_W], bf16, tag="dvesel")
                        nc.vector.tensor_scalar(
                            dve_t[:],
                            iota_t[:],
                            dst_t[:, gb:gb + 1],
                            nrm_t[:, gb:gb + 1],
                            mybir.AluOpType.is_equal,
                            mybir.AluOpType.mult,
                        )
                        s_ap = dve_t[:]
                    else:
                        s_ap = sel_t[:, (b - L) * 128:(b - L + 1) * 128]
                    if b < B_lo:
                        g_ap = lo_block(c * B_lo + b)
                    else:
                        g_ap = hi_block(c * B_hi + (b - B_lo))
                    nc.tensor.matmul(
                        agg_t[:],
                        lhsT=g_ap,
                        rhs=s_ap,
                        start=(b == 0),
                        stop=(b == B - 1),
                    )
                aggs_t = aggsp.tile([128, CHUNK_W], bf16, tag="aggs")
                nc.vector.tensor_copy(aggs_t[:], agg_t[:])
                outp_t = pout.tile([128, CHUNK_W], mybir.dt.float32, tag="outp")
                nc.tensor.matmul(
                    outp_t[:], lhsT=w_t[:], rhs=aggs_t[:], start=True, stop=True
                )
                outs_t = outsp.tile([128, CHUNK_W], bf16, tag="outs")
                nc.scalar.activation(
                    outs_t[:],
                    outp_t[:],
                    mybir.ActivationFunctionType.Relu,
                    bias=bias_t[:, 0:1],
                    scale=1.0,
                )
                out_eng = nc.scalar if c % 2 == 0 else nc.sync
                out_eng.dma_start(out_d[:, c * CHUNK_W:(c + 1) * CHUNK_W], outs_t[:])
    nc.compile()
    return nc


def _make_in_maps(x, weight, bias, per_core, meta):
    xb = np.ascontiguousarray(np.asarray(x, dtype=np.float32)).astype(BF16)
    w = np.ascontiguousarray(np.asarray(weight, dtype=np.float32)).astype(BF16)
    iota = np.tile(np.arange(CHUNK_W, dtype=np.float32), (128, 1)).astype(BF16)
    bvec = np.zeros((128, 1), np.float32)
    bvec[: len(bias), 0] = np.asarray(bias, dtype=np.float32)
    x_lo = np.ascontiguousarray(xb[:SPLIT])
    x_hi = np.ascontiguousarray(xb[SPLIT:])
    in_maps = []
    for k in range(N_CORES):
        pc = per_core[k]
        im = dict(
            x_lo=x_lo,
            x_hi=x_hi,
            idx_lo=pc["idx_lo"],
            idx_hi=pc["idx_hi"],
            seldata=pc["seldata"],
            weight=w,
            bias=bvec,
        )
        if pc["dstslot"].size:
            im.update(dstslot=pc["dstslot"], normv=pc["normv"], iota=iota)
        in_maps.append(im)
    return in_maps


def _unshard(results, meta):
    outs = [np.asarray(results[k]["out"], dtype=np.float32) for k in range(N_CORES)]
    big = np.concatenate(outs, axis=1).reshape(128, meta["nchunks"], CHUNK_W)
    return np.ascontiguousarray(big[:, meta["chunk_of"], meta["slot_of"]].T)


def kernel(x, edge_index, weight, bias):
    x = np.asarray(x)
    per_core, meta = _host_prep(x, edge_index)
    nc = _build_program(meta["N"], meta["C"], meta["B_lo"], meta["B_hi"], meta["cpc"])
    in_maps = _make_in_maps(x, np.asarray(weight), np.asarray(bias), per_core, meta)
    res = run_bass_kernel_spmd(
        nc,
        in_maps,
        list(range(N_CORES)),
        trace=os.environ.get("GCN_TRACE", "0") == "1",
    )
    LAST_RUN_INFO["exec_time_ns"] = res.exec_time_ns
    LAST_RUN_INFO["meta"] = {k: v for k, v in meta.items() if np.isscalar(v)}
    return _unshard(res.results, meta)


# revision 63
# speedup vs baseline: 1.0322x; 1.0322x over previous
"""GCN layer (GCNConv + ReLU) Bass kernel for 8 Trainium2 NeuronCores. v2

Reference computation (PyG GCNConv with self-loops, eval mode):
    deg  = in-degree(dst) + 1                       (self loops included)
    norm_e = deg^-1/2[src_e] * deg^-1/2[dst_e]
    out  = relu( segment_sum_dst( x[src] * norm ) @ W + b )   (W applied last)

Device strategy (per core, SPMD over 8 cores):
  - dst nodes are greedily packed into chunks of <=125 nodes (128 slots),
    balancing BOTH the lo-src (src<32768) and hi-src in-edge counts so every
    chunk fits B_LO lo blocks + B_HI hi blocks of 128 edges.
  - x is stored in DRAM in bf16 (lo/hi split for the int16 gather index
    limit). Per 1024 edges one SWDGE dma_gather pulls rows into SBUF;
    gathers round-robin over 4 SWDGE queues (num_swdge_queues=4 plus a 64KB
    descriptor-ring carveout) so descriptor generation runs on all 4 Q7
    core pairs concurrently: ~2.2ns/idx vs ~8.5ns/idx on queue 0 alone.
  - The select matrices S[e, slot] = onehot(dst_e)*norm_e are built on the
    HOST (bf16, one [128, B*128] supertile per chunk) and streamed in by
    HWDGE DMA; keeping the DVE idle matters because DVE activity contends
    with Q7 descriptor generation for the shared SBUF port.  TensorE
    accumulates agg[cin, slot] += sum_e G[e, cin] * S[e, slot] in PSUM.
  - Per chunk: aggT @ W (bf16), bias+ReLU on ScalarE, store bf16
    [cout, slot]; host unpermutes/transposes to the full [N, C] f32 output.
"""

import heapq
import os

import ml_dtypes
import numpy as np

import concourse.bacc as bacc
import concourse.bass as bass
import concourse.mybir as mybir
import concourse.tile as tile
from concourse.bass_utils import run_bass_kernel_spmd

N_CORES = 8
CHUNK_W = 128  # dst slots per chunk
SPLIT = 32768  # gather table split point (int16 index limit)
NODES_PER_CHUNK = 125
NQ = 4  # SWDGE queues
# how many select blocks per chunk the DVE builds on-chip (is_equal*norm);
# the rest stream from host-built supertiles via DMA
L_DVE = int(os.environ.get("GCN_LDVE", "0"))

LAST_RUN_INFO = {}
BF16 = ml_dtypes.bfloat16


def _host_prep(x, edge_index):
    """Host-side sharding: chunk assignment, edge bucketing, index layout."""
    N, C = x.shape
    src = np.asarray(edge_index[0], dtype=np.int64)
    dst = np.asarray(edge_index[1], dtype=np.int64)
    loops = np.arange(N, dtype=np.int64)
    src = np.concatenate([src, loops])
    dst = np.concatenate([dst, loops])

    deg = np.bincount(dst, minlength=N)
    dinv = (1.0 / np.sqrt(deg.astype(np.float64))).astype(np.float32)
    norm = (dinv[src] * dinv[dst]).astype(np.float32)

    lo_in = np.bincount(dst[src < SPLIT], minlength=N)
    hi_in = deg - lo_in

    nchunks = N_CORES * int(np.ceil(N / (N_CORES * NODES_PER_CHUNK)))
    cpc = nchunks // N_CORES

    # Greedy bi-criteria packing: balance lo and hi edge counts per chunk.
    chunk_of, load_lo, load_hi = _greedy_chunks(lo_in, hi_in, nchunks)
    # swap-repair towards the block-quantized ideal (saves a whole gather
    # block per chunk when the greedy max is just above a 128 boundary)
    t_lo = max(int(np.ceil(load_lo.sum() / nchunks / 128)) * 128, 128)
    t_hi = max(int(np.ceil(load_hi.sum() / nchunks / 128)) * 128, 128)
    if load_lo.max() > t_lo or load_hi.max() > t_hi:
        rep = _repair_chunks(chunk_of, load_lo, load_hi, lo_in, hi_in, t_lo, t_hi)
        if rep is not None:
            chunk_of, load_lo, load_hi = rep
    B_lo = int(np.ceil(load_lo.max() / 128))
    B_hi = int(np.ceil(load_hi.max() / 128))
    B = B_lo + B_hi

    # slot within chunk
    order = np.argsort(chunk_of, kind="stable")
    slot_of = np.empty(N, np.int64)
    slot_of[order] = np.arange(N) - np.searchsorted(
        chunk_of[order], chunk_of[order], side="left"
    )
    assert slot_of.max() < CHUNK_W

    # bucket edges by (chunk, hi) and rank within bucket
    e_chunk = chunk_of[dst]
    islo = src < SPLIT
    key = e_chunk * 2 + (~islo).astype(np.int64)
    perm = np.argsort(key, kind="stable")
    ks = key[perm]
    gsz = np.bincount(key, minlength=2 * nchunks)
    gstart = np.zeros(2 * nchunks, np.int64)
    gstart[1:] = np.cumsum(gsz)[:-1]
    rank = np.arange(len(ks)) - gstart[ks]

    S_lo, S_hi = B_lo * 128, B_hi * 128
    # per-chunk flat edge slots: lo region then hi region
    flat_idx = np.zeros(nchunks * (S_lo + S_hi), np.int64)  # pad idx -> 0
    flat_dst = np.full(nchunks * (S_lo + S_hi), -1.0, np.float32)
    flat_nrm = np.zeros(nchunks * (S_lo + S_hi), np.float32)
    cc = ks // 2
    s = ks % 2
    col = cc * (S_lo + S_hi) + np.where(s == 0, rank, S_lo + rank)
    ss = src[perm]
    flat_idx[col] = np.where(s == 0, ss, ss - SPLIT)
    flat_dst[col] = slot_of[dst[perm]].astype(np.float32)
    flat_nrm[col] = norm[perm]
    assert flat_idx.max() < SPLIT and flat_idx.min() >= 0
    flat_idx = flat_idx.astype(np.int16)

    A_idx = flat_idx.reshape(nchunks, S_lo + S_hi)
    A_dst = flat_dst.reshape(nchunks, (S_lo + S_hi) // 128, 128)
    A_nrm = flat_nrm.reshape(nchunks, (S_lo + S_hi) // 128, 128)

    # Select matrices: blocks [0, L) of each chunk are built on-chip by the
    # DVE from (dstslot, norm) scalars; blocks [L, B) are host-built bf16
    # supertiles S[p, (chunk*(B-L) + block-L)*128 + dstslot] = norm.
    L = min(L_DVE, B)
    ch_i, blk_i, p_i = np.nonzero(A_dst >= 0)
    dma_m = blk_i >= L
    slot_i = A_dst[ch_i, blk_i, p_i].astype(np.int64)
    nrm_i = A_nrm[ch_i, blk_i, p_i].astype(BF16)
    col_i = (ch_i % cpc) * ((B - L) * 128) + (blk_i - L) * 128 + slot_i

    per_core = []
    for k in range(N_CORES):
        sl = slice(k * cpc, (k + 1) * cpc)
        ilo = A_idx[sl, :S_lo].reshape(-1)  # cpc*B_lo*128, stream order
        ihi = A_idx[sl, S_lo:].reshape(-1)
        m = (ch_i >= k * cpc) & (ch_i < (k + 1) * cpc) & dma_m
        seldata = np.zeros((128, cpc * (B - L) * 128), BF16)
        seldata[p_i[m], col_i[m]] = nrm_i[m]
        per_core.append(
            dict(
                idx_lo=_wrap_idx(ilo),
                idx_hi=_wrap_idx(ihi),
                seldata=seldata,
                dstslot=np.ascontiguousarray(
                    A_dst[sl, :L, :].reshape(cpc * L, 128).T
                ),
                normv=np.ascontiguousarray(
                    A_nrm[sl, :L, :].reshape(cpc * L, 128).T
                ),
            )
        )

    meta = dict(
        N=N,
        C=C,
        B_lo=B_lo,
        B_hi=B_hi,
        cpc=cpc,
        nchunks=nchunks,
        chunk_of=chunk_of,
        slot_of=slot_of,
    )
    return per_core, meta


def _greedy_chunks(lo_in, hi_in, nchunks):
    """Pack nodes into chunks (<=NODES_PER_CHUNK each), balancing lo and hi
    in-edge loads.  Returns chunk assignment and per-chunk loads."""
    mean_lo = lo_in.sum() / nchunks
    mean_hi = hi_in.sum() / nchunks
    for slack_lo, slack_hi in [
        (1.058, 1.046),
        (1.058, 1.14),
        (1.10, 1.22),
        (1.25, 1.40),
        (9.9, 9.9),
    ]:
        cap_lo = int(mean_lo * slack_lo)
        cap_hi = int(mean_hi * slack_hi)
        res = _greedy_try(lo_in, hi_in, nchunks, cap_lo, cap_hi)
        if res is not None:
            return res
    raise RuntimeError("chunk packing failed")


def _repair_chunks(chunk_of, load_lo, load_hi, lo_in, hi_in, t_lo, t_hi):
    """Local-search repair: swap nodes between chunks until every chunk's
    lo load <= t_lo and hi load <= t_hi.  Returns None if stuck (caller
    keeps the unrepaired assignment)."""
    if load_lo.sum() > t_lo * len(load_lo) or load_hi.sum() > t_hi * len(load_hi):
        return None  # targets infeasible
    chunk_of = chunk_of.copy()
    load_lo = load_lo.copy()
    load_hi = load_hi.copy()
    nch = len(load_lo)
    members = [[] for _ in range(nch)]
    for n in range(len(chunk_of)):
        members[chunk_of[n]].append(n)

    def fix(dim_in, other_in, load_d, load_o, t_d, t_o):
        for A in np.argsort(-load_d):
            A = int(A)
            guard = 0
            while load_d[A] > t_d and guard < 400:
                guard += 1
                a = max(members[A], key=lambda n: dim_in[n])
                done = False
                for B in np.argsort(load_d):
                    B = int(B)
                    if B == A:
                        continue
                    for b in sorted(members[B], key=lambda n: dim_in[n])[:24]:
                        ndA = load_d[A] - dim_in[a] + dim_in[b]
                        noA = load_o[A] - other_in[a] + other_in[b]
                        ndB = load_d[B] - dim_in[b] + dim_in[a]
                        noB = load_o[B] - other_in[b] + other_in[a]
                        if (
                            ndA < load_d[A]
                            and noA <= t_o
                            and ndB <= t_d
                            and noB <= t_o
                        ):
                            members[A].remove(a)
                            members[B].remove(b)
                            members[A].append(b)
                            members[B].append(a)
                            chunk_of[a], chunk_of[b] = B, A
                            load_d[A], load_o[A] = ndA, noA
                            load_d[B], load_o[B] = ndB, noB
                            done = True
                            break
                    if done:
                        break
                if not done:
                    return False
        return True

    if not fix(lo_in, hi_in, load_lo, load_hi, t_lo, t_hi):
        return None
    if not fix(hi_in, lo_in, load_hi, load_lo, t_hi, t_lo):
        return None
    if load_lo.max() > t_lo or load_hi.max() > t_hi:
        return None
    return chunk_of, load_lo, load_hi


def _greedy_try(lo_in, hi_in, nchunks, cap_lo, cap_hi):
    N = len(lo_in)
    order = np.argsort(-(lo_in / cap_lo + hi_in / cap_hi), kind="stable")
    load_lo = np.zeros(nchunks)
    load_hi = np.zeros(nchunks)
    cnt = np.zeros(nchunks, np.int64)
    chunk_of = np.empty(N, np.int64)
    h = [(0.0, c) for c in range(nchunks)]
    heapq.heapify(h)
    for n in order:
        popped = []
        while True:
            if not h:
                return None
            l, c = heapq.heappop(h)
            if (
                cnt[c] < NODES_PER_CHUNK
                and load_lo[c] + lo_in[n] <= cap_lo
                and load_hi[c] + hi_in[n] <= cap_hi
            ):
                break
            popped.append((l, c))
        for p in popped:
            heapq.heappush(h, p)
        chunk_of[n] = c
        cnt[c] += 1
        load_lo[c] += lo_in[n]
        load_hi[c] += hi_in[n]
        heapq.heappush(h, (max(load_lo[c] / cap_lo, load_hi[c] / cap_hi), c))
    return chunk_of, load_lo, load_hi


def _wrap_idx(flat):
    """Gather idx layout: per 1024-idx instruction, [i%16 (x8 replicated), i//16].
    flat is the concatenated block stream; padded to a multiple of 1024 with
    -1: the gather ucode trims trailing negative indices before descriptor
    generation, and these slots belong to no chunk so they are never read."""
    n = len(flat)
    ninst = int(np.ceil(n / 1024))
    pad = np.full(ninst * 1024, -1, flat.dtype)
    pad[:n] = flat
    cols = []
    for i in range(ninst):
        v = pad[i * 1024:(i + 1) * 1024].reshape(64, 16).T  # [16, 64]
        cols.append(np.tile(v, (8, 1)))
    return np.ascontiguousarray(np.concatenate(cols, axis=1))


def _build_program(N, C, B_lo, B_hi, cpc):
    f32 = mybir.dt.float32
    bf16 = mybir.dt.bfloat16
    i16 = mybir.dt.int16
    B = B_lo + B_hi
    L = min(L_DVE, B)
    n_lo_inst = (cpc * B_lo * 128 + 1023) // 1024
    n_hi_inst = (cpc * B_hi * 128 + 1023) // 1024

    nc = bacc.Bacc(None, target_bir_lowering=False, debug=False,
                   num_swdge_queues=NQ, dynamic_dma_scratch_size=65536)

    xlo_d = nc.dram_tensor("x_lo", [SPLIT, C], bf16, kind="ExternalInput")
    xhi_d = nc.dram_tensor("x_hi", [N - SPLIT, C], bf16, kind="ExternalInput")
    ilo_d = nc.dram_tensor("idx_lo", [128, n_lo_inst * 64], i16, kind="ExternalInput")
    ihi_d = nc.dram_tensor("idx_hi", [128, n_hi_inst * 64], i16, kind="ExternalInput")
    sel_d = nc.dram_tensor("seldata", [128, cpc * (B - L) * 128], bf16, kind="ExternalInput")
    if L:
        dst_d = nc.dram_tensor("dstslot", [128, cpc * L], f32, kind="ExternalInput")
        nrm_d = nc.dram_tensor("normv", [128, cpc * L], f32, kind="ExternalInput")
        iota_d = nc.dram_tensor("iota", [128, CHUNK_W], bf16, kind="ExternalInput")
    w_d = nc.dram_tensor("weight", [C, C], bf16, kind="ExternalInput")
    b_d = nc.dram_tensor("bias", [128, 1], f32, kind="ExternalInput")
    out_d = nc.dram_tensor("out", [128, cpc * CHUNK_W], bf16, kind="ExternalOutput")

    with tile.TileContext(nc) as tc:
        with (
            tc.tile_pool(name="const", bufs=1) as constp,
            tc.tile_pool(name="gat_lo", bufs=12) as glop,
            tc.tile_pool(name="gat_hi", bufs=8) as ghip,
            tc.tile_pool(name="sel", bufs=6) as selp,
            tc.tile_pool(name="dvesel", bufs=8) as dvesp,
            tc.tile_pool(name="aggs", bufs=4) as aggsp,
            tc.tile_pool(name="outs", bufs=4) as outsp,
            tc.tile_pool(name="pagg", bufs=4, space="PSUM") as pagg,
            tc.tile_pool(name="pout", bufs=2, space="PSUM") as pout,
        ):
            # idx tables first: the first real gather depends only on these
            ilo_t = constp.tile([128, n_lo_inst * 64], i16, tag="ilo")
            nc.sync.dma_start(ilo_t[:], ilo_d[:])
            ihi_t = constp.tile([128, n_hi_inst * 64], i16, tag="ihi")
            nc.sync.dma_start(ihi_t[:], ihi_d[:])
            # tiny warmup gather: loads the Q7 dma_gather ucode (~6us IRAM
            # load) concurrently with the idx table DMAs
            warm_t = constp.tile([128, 1, C], bf16, tag="warm")
            warmidx_t = constp.tile([128, 8], i16, tag="warmidx")
            nc.gpsimd.memset(warmidx_t[:], 0)
            nc.gpsimd.dma_gather(warm_t[:], xlo_d[:], warmidx_t[:], 128, 128, C)
            w_t = constp.tile([C, C], bf16, tag="w")
            nc.scalar.dma_start(w_t[:], w_d[:])
            bias_t = constp.tile([128, 1], f32, tag="bias")
            nc.scalar.dma_start(bias_t[:], b_d[:])
            if L:
                iota_t = constp.tile([128, CHUNK_W], bf16, tag="iota")
                nc.scalar.dma_start(iota_t[:], iota_d[:])
                dst_t = constp.tile([128, cpc * L], f32, tag="dst")
                nc.scalar.dma_start(dst_t[:], dst_d[:])
                nrm_t = constp.tile([128, cpc * L], f32, tag="nrm")
                nc.scalar.dma_start(nrm_t[:], nrm_d[:])

            qctr = [0]
            lo_tiles, hi_tiles = [], []

            def issue_lo():
                i = len(lo_tiles)
                gt = glop.tile([128, 8, C], bf16, tag="glo")
                nc.gpsimd.dma_gather(
                    gt[:], xlo_d[:], ilo_t[:, i * 64:(i + 1) * 64],
                    1024, 1024, C, queue_num=qctr[0] % NQ,
                )
                qctr[0] += 1
                lo_tiles.append(gt)

            def issue_hi():
                i = len(hi_tiles)
                gt = ghip.tile([128, 8, C], bf16, tag="ghi")
                nc.gpsimd.dma_gather(
                    gt[:], xhi_d[:], ihi_t[:, i * 64:(i + 1) * 64],
                    1024, 1024, C, queue_num=qctr[0] % NQ,
                )
                qctr[0] += 1
                hi_tiles.append(gt)

            def lo_block(g):
                while len(lo_tiles) * 8 <= g:
                    issue_lo()
                return lo_tiles[g // 8][:, g % 8, :]

            def hi_block(g):
                while len(hi_tiles) * 8 <= g:
                    issue_hi()
                return hi_tiles[g // 8][:, g % 8, :]

            for c in range(cpc):
                agg_t = pagg.tile([128, CHUNK_W], mybir.dt.float32, tag="agg")
                # split each chunk's supertile across both HWDGE rings so the
                # two halves transfer concurrently
                sel_t = selp.tile([128, (B - L) * 128], bf16, tag="sel")
                half = ((B - L) // 2) * 128
                base = c * (B - L) * 128
                nc.sync.dma_start(
                    sel_t[:, :half], sel_d[:, base:base + half]
                )
                nc.scalar.dma_start(
                    sel_t[:, half:], sel_d[:, base + half:base + (B - L) * 128]
                )
                for b in range(B):
                    if b < L:
                        gb = c * L + b
                        dve_t = dvesp.tile([128, CHUNK_W], bf16, tag="dvesel")
                        nc.vector.tensor_scalar(
                            dve_t[:],
                            iota_t[:],
                            dst_t[:, gb:gb + 1],
                            nrm_t[:, gb:gb + 1],
                            mybir.AluOpType.is_equal,
                            mybir.AluOpType.mult,
                        )
                        s_ap = dve_t[:]
                    else:
                        s_ap = sel_t[:, (b - L) * 128:(b - L + 1) * 128]
                    if b < B_lo:
                        g_ap = lo_block(c * B_lo + b)
                    else:
                        g_ap = hi_block(c * B_hi + (b - B_lo))
                    nc.tensor.matmul(
                        agg_t[:],
                        lhsT=g_ap,
                        rhs=s_ap,
                        start=(b == 0),
                        stop=(b == B - 1),
                    )
                aggs_t = aggsp.tile([128, CHUNK_W], bf16, tag="aggs")
                nc.vector.tensor_copy(aggs_t[:], agg_t[:])
                outp_t = pout.tile([128, CHUNK_W], mybir.dt.float32, tag="outp")
                nc.tensor.matmul(
                    outp_t[:], lhsT=w_t[:], rhs=aggs_t[:], start=True, stop=True
                )
                outs_t = outsp.tile([128, CHUNK_W], bf16, tag="outs")
                nc.scalar.activation(
                    outs_t[:],
                    outp_t[:],
                    mybir.ActivationFunctionType.Relu,
                    bias=bias_t[:, 0:1],
                    scale=1.0,
                )
                out_eng = nc.scalar if c % 2 == 0 else nc.sync
                out_eng.dma_start(out_d[:, c * CHUNK_W:(c + 1) * CHUNK_W], outs_t[:])
    nc.compile()
    return nc


def _make_in_maps(x, weight, bias, per_core, meta):
    xb = np.ascontiguousarray(np.asarray(x, dtype=np.float32)).astype(BF16)
    w = np.ascontiguousarray(np.asarray(weight, dtype=np.float32)).astype(BF16)
    iota = np.tile(np.arange(CHUNK_W, dtype=np.float32), (128, 1)).astype(BF16)
    bvec = np.zeros((128, 1), np.float32)
    bvec[: len(bias), 0] = np.asarray(bias, dtype=np.float32)
    x_lo = np.ascontiguousarray(xb[:SPLIT])
    x_hi = np.ascontiguousarray(xb[SPLIT:])
    in_maps = []
    for k in range(N_CORES):
        pc = per_core[k]
        im = dict(
            x_lo=x_lo,
            x_hi=x_hi,
            idx_lo=pc["idx_lo"],
            idx_hi=pc["idx_hi"],
            seldata=pc["seldata"],
            weight=w,
            bias=bvec,
        )
        if pc["dstslot"].size:
            im.update(dstslot=pc["dstslot"], normv=pc["normv"], iota=iota)
        in_maps.append(im)
    return in_maps


def _unshard(results, meta):
    outs = [np.asarray(results[k]["out"], dtype=np.float32) for k in range(N_CORES)]
    big = np.concatenate(outs, axis=1).reshape(128, meta["nchunks"], CHUNK_W)
    return np.ascontiguousarray(big[:, meta["chunk_of"], meta["slot_of"]].T)


def kernel(x, edge_index, weight, bias):
    x = np.asarray(x)
    per_core, meta = _host_prep(x, edge_index)
    nc = _build_program(meta["N"], meta["C"], meta["B_lo"], meta["B_hi"], meta["cpc"])
    in_maps = _make_in_maps(x, np.asarray(weight), np.asarray(bias), per_core, meta)
    res = run_bass_kernel_spmd(
        nc,
        in_maps,
        list(range(N_CORES)),
        trace=os.environ.get("GCN_TRACE", "0") == "1",
    )
    LAST_RUN_INFO["exec_time_ns"] = res.exec_time_ns
    LAST_RUN_INFO["meta"] = {k: v for k, v in meta.items() if np.isscalar(v)}
    return _unshard(res.results, meta)
